# revision 1
# baseline (speedup 1.0000x reference)
"""Causal (diagonal=1) multi-head-of-one attention for trn2, 8-core SPMD.

Reference computation (fp32):
    k = key @ Wk.T; q = query @ Wq.T; v = value @ Wv.T       # [B,T,H]
    qk = (q @ k.T) / sqrt(E)                                  # [B,T,T]
    qk masked with tril(ones, k=1) and padding_mask           # -inf outside
    attn = softmax(qk, -1) @ v                                # [B,T,H]

Sharding: data-parallel over batch, 2 batches per core, no collectives.

Device kernel (per core, per batch), all matmuls bf16 with fp32 PSUM:
    qT[h,t]  = sum_e WqT[e,h].T-chunks @ queryT[e,t]          (proj)
    kT[h,s]  = likewise; v[s,h] = valueT[e,s].T @ WvT[e,h]
    sT[s,t]  = kT-chunk.T @ qT  (only causally-live s-chunks)
    pT[s,t]  = exp(sT/32)  (ScalarE; max-subtraction skipped: |s/32| <~ 6)
    pT       = affine_select(pT, keep j<=i+1, else 0)         (GPSIMD)
    num[t,h] = pT-chunk.T @ v ; den[t,1] = pT-chunk.T @ pad01
    out[t,h] = num * reciprocal(den)                          (VectorE)

padding_mask is folded in exactly on the host: v rows and the denominator
column are scaled by pad01 = (padding_mask == 0), which equals softmax
with -inf at padded keys.
"""
from contextlib import ExitStack

import numpy as np
import ml_dtypes

import concourse.bass as bass
import concourse.mybir as mybir
import concourse.tile as tile
from concourse.bass_utils import run_bass_kernel_spmd

BF16 = mybir.dt.bfloat16
F32 = mybir.dt.float32
P = 128
T = 1024           # sequence length
E = 1024           # embed dim
H = 1024           # head dim
NB = 16            # full batch
NCORES = 8
BPC = NB // NCORES  # batches per core
NC = T // P        # 128-chunks per dim (8)
SCALE = 1.0 / 32.0  # 1/sqrt(E)

_nc_cache = None


# --- walrus workaround: one sync-wait per instruction ---------------------
def _split_multi_waits(nc):
    """This walrus build rejects instructions with >1 sync wait (2 for
    EventSemaphore).  Move extra waits onto fresh same-engine NOPs placed
    immediately before the instruction; per-engine in-order execution
    preserves the gating, and semaphore updates stay on the original."""
    for fn in nc.m.functions:
        for bb in fn.blocks:
            il = bb.instructions
            idx = 0
            while idx < len(il):
                inst = il[idx]
                si = inst.sync_info
                waits = list(si.on_wait) if si and si.on_wait else []
                cap = 2 if isinstance(inst, mybir.InstEventSemaphore) else 1
                if len(waits) > cap:
                    extra, keep = waits[:-cap], waits[-cap:]
                    for j, w in enumerate(extra):
                        nop = mybir.InstNoOp(
                            name=f"I-wsplit-{inst.name}-{j}",
                            engine=inst.engine,
                            ins=[],
                            outs=[],
                            sync_info=mybir.SyncInfo(on_wait=[w], on_update=[]),
                        )
                        il.insert(idx, nop)
                        idx += 1
                    inst.sync_info = mybir.SyncInfo(
                        on_wait=keep, on_update=list(si.on_update or [])
                    )
                idx += 1


def _n_sc(ti):
    """Number of live 128-wide s-chunks for t-tile ti (cols j <= t+1)."""
    return min(ti + 2, NC)


def _emit_batch(nc, pools, b, dram):
    Exp = mybir.ActivationFunctionType.Exp
    w_q, w_k, w_v = pools["wq"], pools["wk"], pools["wv"]
    sb, ps, psd = pools["sb"], pools["ps"], pools["psd"]

    # -- load inputs + projections, ordered so the first projection's DMAs
    #    issue first and later tensors stream in behind the PE --
    def load_in(tag, dname, interleave=None):
        # interleave: per-ec callback issuing the matching weight-chunk DMA
        # right after the input chunk, so the ec-th matmul's operands arrive
        # together instead of all-weights-then-all-inputs.
        tiles = []
        for ec in range(NC):
            if interleave is not None:
                interleave(ec)
            t = sb.tile([P, T], BF16, name=f"{tag}{ec}")
            nc.sync.dma_start(t[:], dram[dname][b, bass.ts(ec, P), :])
            tiles.append(t)
        return tiles

    qTs = [sb.tile([P, T], BF16, name=f"qTs{h}") for h in range(NC)]
    kTs = [sb.tile([P, T], BF16, name=f"kTs{h}") for h in range(NC)]
    v_sb = [sb.tile([P, T], BF16, name=f"vsb{s}") for s in range(NC)]

    def proj_qk(w_t, x_in, x_out):
        for ht in range(NC):
            for tg in range(2):
                acc = ps.tile([P, 512], F32, name="ps")
                for ec in range(NC):
                    nc.tensor.matmul(
                        acc[:],
                        lhsT=w_t[ec][:, bass.ts(ht, P)],
                        rhs=x_in[ec][:, bass.ts(tg, 512)],
                        start=(ec == 0),
                        stop=(ec == NC - 1),
                    )
                nc.scalar.copy(x_out[ht][:, bass.ts(tg, 512)], acc[:])

    qin = load_in("qin", "qT", interleave=pools.pop("wq_dma", None))
    proj_qk(w_q, qin, qTs)
    kin = load_in("kin", "kT", interleave=pools.pop("wk_dma", None))
    proj_qk(w_k, kin, kTs)
    vin = load_in("vin", "vT", interleave=pools.pop("wv_dma", None))
    padt = sb.tile([P, NC], BF16, name="padt", bufs=2)
    nc.sync.dma_start(
        padt[:], dram["pad"][b].rearrange("(c p) x -> p (c x)", p=P)
    )
    for st in range(NC):
        for hh in range(2):
            acc = ps.tile([P, 512], F32, name="ps")
            for ec in range(NC):
                nc.tensor.matmul(
                    acc[:],
                    lhsT=vin[ec][:, bass.ts(st, P)],
                    rhs=w_v[ec][:, bass.ts(hh, 512)],
                    start=(ec == 0),
                    stop=(ec == NC - 1),
                )
            nc.vector.tensor_copy(v_sb[st][:, bass.ts(hh, 512)], acc[:])

    # -- scores^T + exp + causal zeroing --
    # 256-wide t-groups: computes 46 of the causal-minimum 43 (s,t) block
    # pairs (512-wide groups would compute 52).
    GW = 256
    pT = [sb.tile([P, T], BF16, name=f"pT{s}") for s in range(NC)]
    for g in range(T // GW):
        for sc in range(min((GW * (g + 1)) // P + 1, NC)):
            acc = ps.tile([P, 512], F32, name="ps")
            for hc in range(NC):
                nc.tensor.matmul(
                    acc[:, :GW],
                    lhsT=kTs[hc][:, bass.ts(sc, P)],
                    rhs=qTs[hc][:, bass.ts(g, GW)],
                    start=(hc == 0),
                    stop=(hc == NC - 1),
                )
            dst = pT[sc][:, bass.ts(g, GW)]
            nc.scalar.activation(dst, acc[:, :GW], Exp, scale=SCALE)
            off = 128 * sc - GW * g
            if off >= 0:
                # keep where t_local - s_local - off + 1 >= 0 (j <= i+1)
                nc.gpsimd.affine_select(
                    out=dst,
                    in_=dst,
                    compare_op=mybir.AluOpType.is_ge,
                    fill=0.0,
                    base=1 - off,
                    pattern=[[1, GW]],
                    channel_multiplier=-1,
                )

    # -- attn = (pT.T @ [v, pad01]) with post-normalization --
    for ti in range(NC):
        nsc = _n_sc(ti)
        po0 = ps.tile([P, 512], F32, name="ps")
        po1 = ps.tile([P, 512], F32, name="ps")
        pd = psd.tile([P, 1], F32, name="psd")
        for sc in range(nsc):
            lhsT = pT[sc][:, bass.ts(ti, P)]
            st, sp = (sc == 0), (sc == nsc - 1)
            nc.tensor.matmul(po0[:], lhsT=lhsT, rhs=v_sb[sc][:, 0:512],
                             start=st, stop=sp)
            nc.tensor.matmul(po1[:], lhsT=lhsT, rhs=v_sb[sc][:, 512:1024],
                             start=st, stop=sp)
            nc.tensor.matmul(pd[:], lhsT=lhsT, rhs=padt[:, sc:sc + 1],
                             start=st, stop=sp)
        r = sb.tile([P, 1], F32, name="recip", bufs=3)
        nc.vector.reciprocal(r[:], pd[:])
        osb = sb.tile([P, T], F32, name="osb", bufs=3)
        # the two halves scale concurrently on VectorE and ScalarE
        nc.vector.tensor_scalar_mul(osb[:, 0:512], po0[:], r[:])
        nc.sync.dma_start(dram["out"][b, bass.ts(ti, P), 0:512], osb[:, 0:512])
        nc.scalar.activation(osb[:, 512:1024], po1[:],
                             mybir.ActivationFunctionType.Copy, scale=r[:])
        nc.sync.dma_start(dram["out"][b, bass.ts(ti, P), 512:1024],
                          osb[:, 512:1024])


def _build_nc():
    nc = bass.Bass()
    dram = {
        "qT": nc.declare_dram_parameter("qT", [BPC, E, T], BF16, isOutput=False),
        "kT": nc.declare_dram_parameter("kT", [BPC, E, T], BF16, isOutput=False),
        "vT": nc.declare_dram_parameter("vT", [BPC, E, T], BF16, isOutput=False),
        "wq": nc.declare_dram_parameter("wq", [E, H], BF16, isOutput=False),
        "wk": nc.declare_dram_parameter("wk", [E, H], BF16, isOutput=False),
        "wv": nc.declare_dram_parameter("wv", [E, H], BF16, isOutput=False),
        "pad": nc.declare_dram_parameter("pad", [BPC, T, 1], BF16, isOutput=False),
        "out": nc.declare_dram_parameter("out", [BPC, T, H], F32, isOutput=True),
    }
    with tile.TileContext(nc) as tc, ExitStack() as ctx:
        sb = ctx.enter_context(tc.tile_pool(name="sb", bufs=1))
        ps = ctx.enter_context(tc.tile_pool(name="ps", bufs=6, space="PSUM"))
        psd = ctx.enter_context(tc.tile_pool(name="psd", bufs=2, space="PSUM"))

        pools = {"sb": sb, "ps": ps, "psd": psd}
        for wname in ("wq", "wk", "wv"):
            pools[wname] = [
                sb.tile([P, H], BF16, name=f"{wname}{ec}") for ec in range(NC)
            ]

        def w_dma(wname):
            def go(ec):
                nc.sync.dma_start(
                    pools[wname][ec][:], dram[wname][bass.ts(ec, P), :]
                )
            return go

        # Weight DMAs interleave chunk-by-chunk with batch 0's input loads.
        pools["wq_dma"] = w_dma("wq")
        pools["wk_dma"] = w_dma("wk")
        pools["wv_dma"] = w_dma("wv")

        # PE warm-up: ~3.4us of junk matmuls with no data dependencies fill
        # the startup DMA window and trip the HAM clock gate to 2.4 GHz
        # before the first real matmul arrives.  Output is never read.
        warm = sb.tile([P, 512], BF16, name="warm")
        nc.vector.memset(warm[:], 0.0)
        wps = ps.tile([P, 512], F32, name="ps")
        for _ in range(16):
            nc.tensor.matmul(wps[:], lhsT=warm[:, 0:P], rhs=warm[:],
                             start=True, stop=True)

        for b in range(BPC):
            _emit_batch(nc, pools, b, dram)

    _split_multi_waits(nc)
    return nc


def _get_nc():
    global _nc_cache
    if _nc_cache is None:
        _nc_cache = _build_nc()
    return _nc_cache


def _make_in_maps(key, query, value, padding_mask, Wk, Wq, Wv):
    bf = ml_dtypes.bfloat16
    wq = np.ascontiguousarray(Wq.T).astype(bf)
    wk = np.ascontiguousarray(Wk.T).astype(bf)
    wv = np.ascontiguousarray(Wv.T).astype(bf)
    pad01 = (padding_mask.reshape(NB, T) == 0).astype(np.float32)  # [B,T]
    in_maps = []
    for c in range(NCORES):
        s = slice(BPC * c, BPC * (c + 1))
        qT = np.ascontiguousarray(query[s].transpose(0, 2, 1)).astype(bf)
        kT = np.ascontiguousarray(key[s].transpose(0, 2, 1)).astype(bf)
        vTf = value[s].transpose(0, 2, 1) * pad01[s][:, None, :]
        vT = np.ascontiguousarray(vTf).astype(bf)
        in_maps.append({
            "qT": qT, "kT": kT, "vT": vT,
            "wq": wq, "wk": wk, "wv": wv,
            "pad": pad01[s].astype(bf).reshape(BPC, T, 1),
        })
    return in_maps


def run_on_cores(in_maps, trace=False, **kw):
    nc = _get_nc()
    return run_bass_kernel_spmd(nc, in_maps, list(range(NCORES)), trace=trace, **kw)


def kernel(key, query, value, padding_mask, Wk, Wq, Wv):
    key = np.asarray(key)
    query = np.asarray(query)
    value = np.asarray(value)
    padding_mask = np.asarray(padding_mask)
    in_maps = _make_in_maps(key, query, value, padding_mask,
                            np.asarray(Wk), np.asarray(Wq), np.asarray(Wv))
    res = run_on_cores(in_maps)
    out = np.empty((NB, T, H), np.float32)
    for c in range(NCORES):
        out[BPC * c: BPC * (c + 1)] = res.results[c]["out"]
    return out



# revision 6
# speedup vs baseline: 1.2588x; 1.2588x over previous
"""Causal (diagonal=1) multi-head-of-one attention for trn2, 8-core SPMD.

Reference computation (fp32):
    k = key @ Wk.T; q = query @ Wq.T; v = value @ Wv.T       # [B,T,H]
    qk = (q @ k.T) / sqrt(E)                                  # [B,T,T]
    qk masked with tril(ones, k=1) and padding_mask           # -inf outside
    attn = softmax(qk, -1) @ v                                # [B,T,H]

Sharding: data-parallel over batch, 2 batches per core, no collectives.

The k-projection is folded away on the host: qk = q @ k.T =
query @ (Wq.T @ Wk) @ key.T, so the device multiplies query by the
precomputed W = Wq.T @ Wk and scores directly against the raw key.
This removes 1024^3 MACs per batch (~23% of total PE work).

Device kernel (per core, per batch), all matmuls bf16 with fp32 PSUM:
    tmpT[f,t] = sum_e W[e,f].T-chunks @ queryT[e,t]           (proj)
    v[s,h]    = valueT[e,s].T @ WvT[e,h]
    sT[s,t]   = keyT-chunk.T @ tmpT  (only causally-live s-chunks)
    pT[s,t]   = exp(sT/32)  (ScalarE; max-subtraction skipped: |s/32| <~ 6)
    pT        = affine_select(pT, keep j<=i+1, else 0)        (GPSIMD)
    num[t,h]  = pT-chunk.T @ v ; den[t,1] = pT-chunk.T @ pad01
    out[t,h]  = num * reciprocal(den)                         (VectorE)

padding_mask is folded in exactly on the host: v rows and the denominator
column are scaled by pad01 = (padding_mask == 0), which equals softmax
with -inf at padded keys.
"""
from contextlib import ExitStack

import numpy as np
import ml_dtypes

import concourse.bass as bass
import concourse.mybir as mybir
import concourse.tile as tile
from concourse.bass_utils import run_bass_kernel_spmd

BF16 = mybir.dt.bfloat16
F32 = mybir.dt.float32
P = 128
T = 1024           # sequence length
E = 1024           # embed dim
H = 1024           # head dim
NB = 16            # full batch
NCORES = 8
BPC = NB // NCORES  # batches per core
NC = T // P        # 128-chunks per dim (8)
SCALE = 1.0 / 32.0  # 1/sqrt(E)

_nc_cache = None


# --- walrus workaround: one sync-wait per instruction ---------------------
def _split_multi_waits(nc):
    """This walrus build rejects instructions with >1 sync wait (2 for
    EventSemaphore).  Move extra waits onto fresh same-engine NOPs placed
    immediately before the instruction; per-engine in-order execution
    preserves the gating, and semaphore updates stay on the original."""
    for fn in nc.m.functions:
        for bb in fn.blocks:
            il = bb.instructions
            idx = 0
            while idx < len(il):
                inst = il[idx]
                si = inst.sync_info
                waits = list(si.on_wait) if si and si.on_wait else []
                cap = 2 if isinstance(inst, mybir.InstEventSemaphore) else 1
                if len(waits) > cap:
                    extra, keep = waits[:-cap], waits[-cap:]
                    for j, w in enumerate(extra):
                        nop = mybir.InstNoOp(
                            name=f"I-wsplit-{inst.name}-{j}",
                            engine=inst.engine,
                            ins=[],
                            outs=[],
                            sync_info=mybir.SyncInfo(on_wait=[w], on_update=[]),
                        )
                        il.insert(idx, nop)
                        idx += 1
                    inst.sync_info = mybir.SyncInfo(
                        on_wait=keep, on_update=list(si.on_update or [])
                    )
                idx += 1


def _n_sc(ti):
    """Number of live 128-wide s-chunks for t-tile ti (cols j <= t+1)."""
    return min(ti + 2, NC)


def _emit_batch(nc, pools, b, dram):
    Exp = mybir.ActivationFunctionType.Exp
    w_q, w_v = pools["wq"], pools["wv"]
    sb, ps, psd = pools["sb"], pools["ps"], pools["psd"]

    # -- load inputs + projections, ordered so the first projection's DMAs
    #    issue first and later tensors stream in behind the PE --
    def load_in(tag, dname, interleave=None):
        # interleave: per-ec callback issuing the matching weight-chunk DMA
        # right after the input chunk, so the ec-th matmul's operands arrive
        # together instead of all-weights-then-all-inputs.
        tiles = []
        for ec in range(NC):
            if interleave is not None:
                interleave(ec)
            t = sb.tile([P, T], BF16, name=f"{tag}{ec}")
            nc.sync.dma_start(t[:], dram[dname][b, bass.ts(ec, P), :])
            tiles.append(t)
        return tiles

    qTs = [sb.tile([P, T], BF16, name=f"qTs{h}") for h in range(NC)]
    v_sb = [sb.tile([P, T], BF16, name=f"vsb{s}") for s in range(NC)]

    def proj_qk(w_t, x_in, x_out):
        for ht in range(NC):
            for tg in range(2):
                acc = ps.tile([P, 512], F32, name="ps")
                for ec in range(NC):
                    nc.tensor.matmul(
                        acc[:],
                        lhsT=w_t[ec][:, bass.ts(ht, P)],
                        rhs=x_in[ec][:, bass.ts(tg, 512)],
                        start=(ec == 0),
                        stop=(ec == NC - 1),
                    )
                nc.scalar.copy(x_out[ht][:, bass.ts(tg, 512)], acc[:])

    qin = load_in("qin", "qT", interleave=pools.pop("wq_dma", None))
    proj_qk(w_q, qin, qTs)
    # raw key^T chunks feed the score matmuls directly (k-proj folded into W)
    kTs = load_in("kTs", "kT", interleave=pools.pop("wv_dma", None))
    vin = load_in("vin", "vT")
    padt = sb.tile([P, NC], BF16, name="padt", bufs=2)
    nc.sync.dma_start(
        padt[:], dram["pad"][b].rearrange("(c p) x -> p (c x)", p=P)
    )
    for st in range(NC):
        for hh in range(2):
            acc = ps.tile([P, 512], F32, name="ps")
            for ec in range(NC):
                nc.tensor.matmul(
                    acc[:],
                    lhsT=vin[ec][:, bass.ts(st, P)],
                    rhs=w_v[ec][:, bass.ts(hh, 512)],
                    start=(ec == 0),
                    stop=(ec == NC - 1),
                )
            nc.vector.tensor_copy(v_sb[st][:, bass.ts(hh, 512)], acc[:])

    # -- scores^T + exp + causal zeroing --
    # 256-wide t-groups: computes 46 of the causal-minimum 43 (s,t) block
    # pairs (512-wide groups would compute 52).
    GW = 256
    pT = [sb.tile([P, T], BF16, name=f"pT{s}") for s in range(NC)]
    for g in range(T // GW):
        for sc in range(min((GW * (g + 1)) // P + 1, NC)):
            acc = ps.tile([P, 512], F32, name="ps")
            for hc in range(NC):
                nc.tensor.matmul(
                    acc[:, :GW],
                    lhsT=kTs[hc][:, bass.ts(sc, P)],
                    rhs=qTs[hc][:, bass.ts(g, GW)],
                    start=(hc == 0),
                    stop=(hc == NC - 1),
                )
            dst = pT[sc][:, bass.ts(g, GW)]
            nc.scalar.activation(dst, acc[:, :GW], Exp, scale=SCALE)
            off = 128 * sc - GW * g
            if off >= 0:
                # keep where t_local - s_local - off + 1 >= 0 (j <= i+1)
                nc.gpsimd.affine_select(
                    out=dst,
                    in_=dst,
                    compare_op=mybir.AluOpType.is_ge,
                    fill=0.0,
                    base=1 - off,
                    pattern=[[1, GW]],
                    channel_multiplier=-1,
                )

    # -- attn = (pT.T @ [v, pad01]) with post-normalization --
    for ti in range(NC):
        nsc = _n_sc(ti)
        po0 = ps.tile([P, 512], F32, name="ps")
        po1 = ps.tile([P, 512], F32, name="ps")
        pd = psd.tile([P, 1], F32, name="psd")
        for sc in range(nsc):
            lhsT = pT[sc][:, bass.ts(ti, P)]
            st, sp = (sc == 0), (sc == nsc - 1)
            nc.tensor.matmul(po0[:], lhsT=lhsT, rhs=v_sb[sc][:, 0:512],
                             start=st, stop=sp)
            nc.tensor.matmul(po1[:], lhsT=lhsT, rhs=v_sb[sc][:, 512:1024],
                             start=st, stop=sp)
            nc.tensor.matmul(pd[:], lhsT=lhsT, rhs=padt[:, sc:sc + 1],
                             start=st, stop=sp)
        r = sb.tile([P, 1], F32, name="recip", bufs=3)
        nc.vector.reciprocal(r[:], pd[:])
        osb = sb.tile([P, T], F32, name="osb", bufs=3)
        # the two halves scale concurrently on VectorE and ScalarE
        nc.vector.tensor_scalar_mul(osb[:, 0:512], po0[:], r[:])
        nc.sync.dma_start(dram["out"][b, bass.ts(ti, P), 0:512], osb[:, 0:512])
        nc.scalar.activation(osb[:, 512:1024], po1[:],
                             mybir.ActivationFunctionType.Copy, scale=r[:])
        nc.sync.dma_start(dram["out"][b, bass.ts(ti, P), 512:1024],
                          osb[:, 512:1024])


def _build_nc():
    nc = bass.Bass()
    dram = {
        "qT": nc.declare_dram_parameter("qT", [BPC, E, T], BF16, isOutput=False),
        "kT": nc.declare_dram_parameter("kT", [BPC, E, T], BF16, isOutput=False),
        "vT": nc.declare_dram_parameter("vT", [BPC, E, T], BF16, isOutput=False),
        # "wq" holds W = Wq.T @ Wk (k-proj folded on host)
        "wq": nc.declare_dram_parameter("wq", [E, H], BF16, isOutput=False),
        "wv": nc.declare_dram_parameter("wv", [E, H], BF16, isOutput=False),
        "pad": nc.declare_dram_parameter("pad", [BPC, T, 1], BF16, isOutput=False),
        "out": nc.declare_dram_parameter("out", [BPC, T, H], F32, isOutput=True),
    }
    with tile.TileContext(nc) as tc, ExitStack() as ctx:
        sb = ctx.enter_context(tc.tile_pool(name="sb", bufs=1))
        ps = ctx.enter_context(tc.tile_pool(name="ps", bufs=6, space="PSUM"))
        psd = ctx.enter_context(tc.tile_pool(name="psd", bufs=2, space="PSUM"))

        pools = {"sb": sb, "ps": ps, "psd": psd}
        for wname in ("wq", "wv"):
            pools[wname] = [
                sb.tile([P, H], BF16, name=f"{wname}{ec}") for ec in range(NC)
            ]

        def w_dma(wname):
            def go(ec):
                nc.sync.dma_start(
                    pools[wname][ec][:], dram[wname][bass.ts(ec, P), :]
                )
            return go

        # Weight DMAs interleave chunk-by-chunk with batch 0's input loads.
        pools["wq_dma"] = w_dma("wq")
        pools["wv_dma"] = w_dma("wv")

        # PE warm-up: ~3.4us of junk matmuls with no data dependencies fill
        # the startup DMA window and trip the HAM clock gate to 2.4 GHz
        # before the first real matmul arrives.  Output is never read.
        warm = sb.tile([P, 512], BF16, name="warm")
        nc.vector.memset(warm[:], 0.0)
        wps = ps.tile([P, 512], F32, name="ps")
        for _ in range(16):
            nc.tensor.matmul(wps[:], lhsT=warm[:, 0:P], rhs=warm[:],
                             start=True, stop=True)

        for b in range(BPC):
            _emit_batch(nc, pools, b, dram)

    _split_multi_waits(nc)
    return nc


def _get_nc():
    global _nc_cache
    if _nc_cache is None:
        _nc_cache = _build_nc()
    return _nc_cache


def _make_in_maps(key, query, value, padding_mask, Wk, Wq, Wv):
    bf = ml_dtypes.bfloat16
    # Fold the k-projection into the q side: q @ k.T = query @ W @ key.T
    W = (Wq.astype(np.float64).T @ Wk.astype(np.float64)).astype(np.float32)
    wq = np.ascontiguousarray(W).astype(bf)  # [E, E]
    wv = np.ascontiguousarray(Wv.T).astype(bf)
    pad01 = (padding_mask.reshape(NB, T) == 0).astype(np.float32)  # [B,T]
    in_maps = []
    for c in range(NCORES):
        s = slice(BPC * c, BPC * (c + 1))
        qT = np.ascontiguousarray(query[s].transpose(0, 2, 1)).astype(bf)
        kT = np.ascontiguousarray(key[s].transpose(0, 2, 1)).astype(bf)
        vTf = value[s].transpose(0, 2, 1) * pad01[s][:, None, :]
        vT = np.ascontiguousarray(vTf).astype(bf)
        in_maps.append({
            "qT": qT, "kT": kT, "vT": vT,
            "wq": wq, "wv": wv,
            "pad": pad01[s].astype(bf).reshape(BPC, T, 1),
        })
    return in_maps


def run_on_cores(in_maps, trace=False, **kw):
    nc = _get_nc()
    return run_bass_kernel_spmd(nc, in_maps, list(range(NCORES)), trace=trace, **kw)


def kernel(key, query, value, padding_mask, Wk, Wq, Wv):
    key = np.asarray(key)
    query = np.asarray(query)
    value = np.asarray(value)
    padding_mask = np.asarray(padding_mask)
    in_maps = _make_in_maps(key, query, value, padding_mask,
                            np.asarray(Wk), np.asarray(Wq), np.asarray(Wv))
    res = run_on_cores(in_maps)
    out = np.empty((NB, T, H), np.float32)
    for c in range(NCORES):
        out[BPC * c: BPC * (c + 1)] = res.results[c]["out"]
    return out



# revision 13
# speedup vs baseline: 1.2766x; 1.0141x over previous
"""Causal (diagonal=1) multi-head-of-one attention for trn2, 8-core SPMD.

Reference computation (fp32):
    k = key @ Wk.T; q = query @ Wq.T; v = value @ Wv.T       # [B,T,H]
    qk = (q @ k.T) / sqrt(E)                                  # [B,T,T]
    qk masked with tril(ones, k=1) and padding_mask           # -inf outside
    attn = softmax(qk, -1) @ v                                # [B,T,H]

Sharding: data-parallel over batch, 2 batches per core, no collectives.

The k-projection is folded away on the host: qk = q @ k.T =
query @ (Wq.T @ Wk) @ key.T, so the device multiplies query by the
precomputed W = Wq.T @ Wk and scores directly against the raw key.
This removes 1024^3 MACs per batch (~23% of total PE work).

Device kernel (per core, per batch), all matmuls bf16 with fp32 PSUM:
    tmpT[f,t] = sum_e W[e,f].T-chunks @ queryT[e,t]           (proj)
    v[s,h]    = valueT[e,s].T @ WvT[e,h]
    sT[s,t]   = keyT-chunk.T @ tmpT  (only causally-live s-chunks)
    pT[s,t]   = exp(sT/32)  (ScalarE; max-subtraction skipped: |s/32| <~ 6)
    pT        = affine_select(pT, keep j<=i+1, else 0)        (GPSIMD)
    num[t,h]  = pT-chunk.T @ v ; den[t,1] = pT-chunk.T @ pad01
    out[t,h]  = num * reciprocal(den)                         (VectorE)

padding_mask is folded in exactly on the host: v rows and the denominator
column are scaled by pad01 = (padding_mask == 0), which equals softmax
with -inf at padded keys.
"""
from contextlib import ExitStack

import numpy as np
import ml_dtypes

import concourse.bass as bass
import concourse.mybir as mybir
import concourse.tile as tile
from concourse.bass_utils import run_bass_kernel_spmd

BF16 = mybir.dt.bfloat16
F32 = mybir.dt.float32
P = 128
T = 1024           # sequence length
E = 1024           # embed dim
H = 1024           # head dim
NB = 16            # full batch
NCORES = 8
BPC = NB // NCORES  # batches per core
NC = T // P        # 128-chunks per dim (8)
SCALE = 1.0 / 32.0  # 1/sqrt(E)

_nc_cache = None


# --- walrus workaround: one sync-wait per instruction ---------------------
def _split_multi_waits(nc):
    """This walrus build rejects instructions with >1 sync wait (2 for
    EventSemaphore).  Move extra waits onto fresh same-engine NOPs placed
    immediately before the instruction; per-engine in-order execution
    preserves the gating, and semaphore updates stay on the original."""
    for fn in nc.m.functions:
        for bb in fn.blocks:
            il = bb.instructions
            idx = 0
            while idx < len(il):
                inst = il[idx]
                si = inst.sync_info
                waits = list(si.on_wait) if si and si.on_wait else []
                cap = 2 if isinstance(inst, mybir.InstEventSemaphore) else 1
                if len(waits) > cap:
                    extra, keep = waits[:-cap], waits[-cap:]
                    for j, w in enumerate(extra):
                        nop = mybir.InstNoOp(
                            name=f"I-wsplit-{inst.name}-{j}",
                            engine=inst.engine,
                            ins=[],
                            outs=[],
                            sync_info=mybir.SyncInfo(on_wait=[w], on_update=[]),
                        )
                        il.insert(idx, nop)
                        idx += 1
                    inst.sync_info = mybir.SyncInfo(
                        on_wait=keep, on_update=list(si.on_update or [])
                    )
                idx += 1


def _n_sc(ti):
    """Number of live 128-wide s-chunks for t-tile ti (cols j <= t+1)."""
    return min(ti + 2, NC)


def _emit_batch(nc, pools, b, dram):
    Exp = mybir.ActivationFunctionType.Exp
    w_q, w_v = pools["wq"], pools["wv"]
    sb, ps, psd = pools["sb"], pools["ps"], pools["psd"]

    # -- load inputs + projections, ordered so the first projection's DMAs
    #    issue first and later tensors stream in behind the PE --
    def load_in(tag, dname, interleave=None):
        # interleave: per-ec callback issuing the matching weight-chunk DMA
        # right after the input chunk, so the ec-th matmul's operands arrive
        # together instead of all-weights-then-all-inputs.
        tiles = []
        for ec in range(NC):
            if interleave is not None:
                interleave(ec)
            t = sb.tile([P, T], BF16, name=f"{tag}{ec}")
            nc.sync.dma_start(t[:], dram[dname][b, bass.ts(ec, P), :])
            tiles.append(t)
        return tiles

    qTs = [sb.tile([P, T], BF16, name=f"qTs{h}") for h in range(NC)]
    v_sb = [sb.tile([P, T], BF16, name=f"vsb{s}") for s in range(NC)]

    def proj_qk(w_t, x_in, x_out):
        for ht in range(NC):
            for tg in range(2):
                acc = ps.tile([P, 512], F32, name="ps")
                for ec in range(NC):
                    nc.tensor.matmul(
                        acc[:],
                        lhsT=w_t[ec][:, bass.ts(ht, P)],
                        rhs=x_in[ec][:, bass.ts(tg, 512)],
                        start=(ec == 0),
                        stop=(ec == NC - 1),
                    )
                nc.scalar.copy(x_out[ht][:, bass.ts(tg, 512)], acc[:])

    qin = load_in("qin", "qT", interleave=pools.pop("wq_dma", None))
    proj_qk(w_q, qin, qTs)
    # raw key^T chunks feed the score matmuls directly (k-proj folded into W)
    kTs = load_in("kTs", "kT", interleave=pools.pop("wv_dma", None))
    vin = load_in("vin", "vT")
    padt = sb.tile([P, NC], BF16, name="padt", bufs=2)
    nc.sync.dma_start(
        padt[:], dram["pad"][b].rearrange("(c p) x -> p (c x)", p=P)
    )
    for st in range(NC):
        for hh in range(2):
            acc = ps.tile([P, 512], F32, name="ps")
            for ec in range(NC):
                nc.tensor.matmul(
                    acc[:],
                    lhsT=vin[ec][:, bass.ts(st, P)],
                    rhs=w_v[ec][:, bass.ts(hh, 512)],
                    start=(ec == 0),
                    stop=(ec == NC - 1),
                )
            nc.vector.tensor_copy(v_sb[st][:, bass.ts(hh, 512)], acc[:])

    # -- scores^T + exp + causal zeroing --
    # 256-wide t-groups; the straddle block (off == 256) is computed at
    # half width (only its second t-half is causally live), which brings
    # the block count to the causal minimum 43.
    GW = 256
    pT = [sb.tile([P, T], BF16, name=f"pT{s}") for s in range(NC)]
    for g in range(T // GW):
        for sc in range(min((GW * (g + 1)) // P + 1, NC)):
            off = 128 * sc - GW * g
            # column window of this t-group that is causally live
            c0 = 128 if off >= 256 else 0
            cw = GW - c0
            acc = ps.tile([P, 512], F32, name="ps")
            for hc in range(NC):
                nc.tensor.matmul(
                    acc[:, :cw],
                    lhsT=kTs[hc][:, bass.ts(sc, P)],
                    rhs=qTs[hc][:, GW * g + c0: GW * g + GW],
                    start=(hc == 0),
                    stop=(hc == NC - 1),
                )
            dst = pT[sc][:, GW * g + c0: GW * g + GW]
            nc.scalar.activation(dst, acc[:, :cw], Exp, scale=SCALE)
            if off >= 0:
                # keep where t_local - s_local - (off-c0) + 1 >= 0 (j <= i+1)
                nc.gpsimd.affine_select(
                    out=dst,
                    in_=dst,
                    compare_op=mybir.AluOpType.is_ge,
                    fill=0.0,
                    base=1 - (off - c0),
                    pattern=[[1, cw]],
                    channel_multiplier=-1,
                )

    # -- attn = (pT.T @ [v, pad01]) with post-normalization --
    for ti in range(NC):
        nsc = _n_sc(ti)
        po0 = ps.tile([P, 512], F32, name="ps")
        po1 = ps.tile([P, 512], F32, name="ps")
        pd = psd.tile([P, 1], F32, name="psd")
        for sc in range(nsc):
            lhsT = pT[sc][:, bass.ts(ti, P)]
            st, sp = (sc == 0), (sc == nsc - 1)
            # den (N=1) sits between the two wide matmuls so its ldweights
            # and po1's both hide under po0's 512-cycle moving phase
            nc.tensor.matmul(po0[:], lhsT=lhsT, rhs=v_sb[sc][:, 0:512],
                             start=st, stop=sp)
            nc.tensor.matmul(pd[:], lhsT=lhsT, rhs=padt[:, sc:sc + 1],
                             start=st, stop=sp)
            nc.tensor.matmul(po1[:], lhsT=lhsT, rhs=v_sb[sc][:, 512:1024],
                             start=st, stop=sp)
        r = sb.tile([P, 1], F32, name="recip", bufs=3)
        nc.vector.reciprocal(r[:], pd[:])
        osb = sb.tile([P, T], BF16, name="osb", bufs=3)
        # the two halves scale concurrently on VectorE and ScalarE
        nc.vector.tensor_scalar_mul(osb[:, 0:512], po0[:], r[:])
        nc.sync.dma_start(dram["out"][b, bass.ts(ti, P), 0:512], osb[:, 0:512])
        nc.scalar.activation(osb[:, 512:1024], po1[:],
                             mybir.ActivationFunctionType.Copy, scale=r[:])
        nc.sync.dma_start(dram["out"][b, bass.ts(ti, P), 512:1024],
                          osb[:, 512:1024])


def _build_nc():
    nc = bass.Bass()
    dram = {
        "qT": nc.declare_dram_parameter("qT", [BPC, E, T], BF16, isOutput=False),
        "kT": nc.declare_dram_parameter("kT", [BPC, E, T], BF16, isOutput=False),
        "vT": nc.declare_dram_parameter("vT", [BPC, E, T], BF16, isOutput=False),
        # "wq" holds W = Wq.T @ Wk (k-proj folded on host)
        "wq": nc.declare_dram_parameter("wq", [E, H], BF16, isOutput=False),
        "wv": nc.declare_dram_parameter("wv", [E, H], BF16, isOutput=False),
        "pad": nc.declare_dram_parameter("pad", [BPC, T, 1], BF16, isOutput=False),
        "z": nc.declare_dram_parameter("z", [P, 512], BF16, isOutput=False),
        "out": nc.declare_dram_parameter("out", [BPC, T, H], BF16, isOutput=True),
    }
    with tile.TileContext(nc) as tc, ExitStack() as ctx:
        sb = ctx.enter_context(tc.tile_pool(name="sb", bufs=1))
        ps = ctx.enter_context(tc.tile_pool(name="ps", bufs=6, space="PSUM"))
        psd = ctx.enter_context(tc.tile_pool(name="psd", bufs=2, space="PSUM"))

        pools = {"sb": sb, "ps": ps, "psd": psd}
        for wname in ("wq", "wv"):
            pools[wname] = [
                sb.tile([P, H], BF16, name=f"{wname}{ec}") for ec in range(NC)
            ]

        def w_dma(wname):
            def go(ec):
                nc.sync.dma_start(
                    pools[wname][ec][:], dram[wname][bass.ts(ec, P), :]
                )
            return go

        # Weight DMAs interleave chunk-by-chunk with batch 0's input loads.
        pools["wq_dma"] = w_dma("wq")
        pools["wv_dma"] = w_dma("wv")

        # PE warm-up: junk matmuls with no compute-side dependencies fill
        # the startup DMA window and trip the HAM clock gate to 2.4 GHz
        # before the first real matmul arrives.  Output is never read.
        # The warm tile is DMA-fed (zeros) so the warmup isn't gated on
        # VectorE sequencer boot + memset.
        warm = sb.tile([P, 512], BF16, name="warm")
        nc.sync.dma_start(warm[:], dram["z"][:, :])
        wps = ps.tile([P, 512], F32, name="ps")
        for _ in range(16):
            nc.tensor.matmul(wps[:], lhsT=warm[:, 0:P], rhs=warm[:],
                             start=True, stop=True)

        for b in range(BPC):
            _emit_batch(nc, pools, b, dram)

    _split_multi_waits(nc)
    return nc


def _get_nc():
    global _nc_cache
    if _nc_cache is None:
        _nc_cache = _build_nc()
    return _nc_cache


def _make_in_maps(key, query, value, padding_mask, Wk, Wq, Wv):
    bf = ml_dtypes.bfloat16
    # Fold the k-projection into the q side: q @ k.T = query @ W @ key.T
    W = (Wq.astype(np.float64).T @ Wk.astype(np.float64)).astype(np.float32)
    wq = np.ascontiguousarray(W).astype(bf)  # [E, E]
    wv = np.ascontiguousarray(Wv.T).astype(bf)
    pad01 = (padding_mask.reshape(NB, T) == 0).astype(np.float32)  # [B,T]
    in_maps = []
    for c in range(NCORES):
        s = slice(BPC * c, BPC * (c + 1))
        qT = np.ascontiguousarray(query[s].transpose(0, 2, 1)).astype(bf)
        kT = np.ascontiguousarray(key[s].transpose(0, 2, 1)).astype(bf)
        vTf = value[s].transpose(0, 2, 1) * pad01[s][:, None, :]
        vT = np.ascontiguousarray(vTf).astype(bf)
        in_maps.append({
            "qT": qT, "kT": kT, "vT": vT,
            "wq": wq, "wv": wv,
            "pad": pad01[s].astype(bf).reshape(BPC, T, 1),
            "z": np.zeros((P, 512), dtype=bf),
        })
    return in_maps


def run_on_cores(in_maps, trace=False, **kw):
    nc = _get_nc()
    return run_bass_kernel_spmd(nc, in_maps, list(range(NCORES)), trace=trace, **kw)


def kernel(key, query, value, padding_mask, Wk, Wq, Wv):
    key = np.asarray(key)
    query = np.asarray(query)
    value = np.asarray(value)
    padding_mask = np.asarray(padding_mask)
    in_maps = _make_in_maps(key, query, value, padding_mask,
                            np.asarray(Wk), np.asarray(Wq), np.asarray(Wv))
    res = run_on_cores(in_maps)
    out = np.empty((NB, T, H), np.float32)
    for c in range(NCORES):
        out[BPC * c: BPC * (c + 1)] = res.results[c]["out"].astype(np.float32)
    return out



# revision 16
# speedup vs baseline: 1.2800x; 1.0027x over previous
"""Causal (diagonal=1) multi-head-of-one attention for trn2, 8-core SPMD.

Reference computation (fp32):
    k = key @ Wk.T; q = query @ Wq.T; v = value @ Wv.T       # [B,T,H]
    qk = (q @ k.T) / sqrt(E)                                  # [B,T,T]
    qk masked with tril(ones, k=1) and padding_mask           # -inf outside
    attn = softmax(qk, -1) @ v                                # [B,T,H]

Sharding: data-parallel over batch, 2 batches per core, no collectives.

The k-projection is folded away on the host: qk = q @ k.T =
query @ (Wq.T @ Wk) @ key.T, so the device multiplies query by the
precomputed W = Wq.T @ Wk and scores directly against the raw key.
This removes 1024^3 MACs per batch (~23% of total PE work).

Device kernel (per core, per batch), all matmuls bf16 with fp32 PSUM:
    tmpT[f,t] = sum_e W[e,f].T-chunks @ queryT[e,t]           (proj)
    v[s,h]    = valueT[e,s].T @ WvT[e,h]
    sT[s,t]   = keyT-chunk.T @ tmpT  (only causally-live s-chunks)
    pT[s,t]   = exp(sT/32)  (ScalarE; max-subtraction skipped: |s/32| <~ 6)
    pT        = affine_select(pT, keep j<=i+1, else 0)        (GPSIMD)
    num[t,h]  = pT-chunk.T @ v ; den[t,1] = pT-chunk.T @ pad01
    out[t,h]  = num * reciprocal(den)                         (VectorE)

padding_mask is folded in exactly on the host: v rows and the denominator
column are scaled by pad01 = (padding_mask == 0), which equals softmax
with -inf at padded keys.
"""
from contextlib import ExitStack

import numpy as np
import ml_dtypes

import concourse.bass as bass
import concourse.mybir as mybir
import concourse.tile as tile
from concourse.bass_utils import run_bass_kernel_spmd

BF16 = mybir.dt.bfloat16
F32 = mybir.dt.float32
P = 128
T = 1024           # sequence length
E = 1024           # embed dim
H = 1024           # head dim
NB = 16            # full batch
NCORES = 8
BPC = NB // NCORES  # batches per core
NC = T // P        # 128-chunks per dim (8)
SCALE = 1.0 / 32.0  # 1/sqrt(E)

_nc_cache = None


# --- walrus workaround: one sync-wait per instruction ---------------------
def _split_multi_waits(nc):
    """This walrus build rejects instructions with >1 sync wait (2 for
    EventSemaphore).  Move extra waits onto fresh same-engine NOPs placed
    immediately before the instruction; per-engine in-order execution
    preserves the gating, and semaphore updates stay on the original."""
    for fn in nc.m.functions:
        for bb in fn.blocks:
            il = bb.instructions
            idx = 0
            while idx < len(il):
                inst = il[idx]
                si = inst.sync_info
                waits = list(si.on_wait) if si and si.on_wait else []
                cap = 2 if isinstance(inst, mybir.InstEventSemaphore) else 1
                if len(waits) > cap:
                    extra, keep = waits[:-cap], waits[-cap:]
                    for j, w in enumerate(extra):
                        nop = mybir.InstNoOp(
                            name=f"I-wsplit-{inst.name}-{j}",
                            engine=inst.engine,
                            ins=[],
                            outs=[],
                            sync_info=mybir.SyncInfo(on_wait=[w], on_update=[]),
                        )
                        il.insert(idx, nop)
                        idx += 1
                    inst.sync_info = mybir.SyncInfo(
                        on_wait=keep, on_update=list(si.on_update or [])
                    )
                idx += 1


def _n_sc(ti):
    """Number of live 128-wide s-chunks for t-tile ti (cols j <= t+1)."""
    return min(ti + 2, NC)


def _emit_batch(nc, pools, b, dram):
    Exp = mybir.ActivationFunctionType.Exp
    w_q, w_v = pools["wq"], pools["wv"]
    sb, ps, psd = pools["sb"], pools["ps"], pools["psd"]

    # -- load inputs + projections, ordered so the first projection's DMAs
    #    issue first and later tensors stream in behind the PE --
    def load_in(tag, dname, interleave=None):
        # interleave: per-ec callback issuing the matching weight-chunk DMA
        # right after the input chunk, so the ec-th matmul's operands arrive
        # together instead of all-weights-then-all-inputs.
        tiles = []
        for ec in range(NC):
            if interleave is not None:
                interleave(ec)
            t = sb.tile([P, T], BF16, name=f"{tag}{ec}")
            nc.sync.dma_start(t[:], dram[dname][b, bass.ts(ec, P), :])
            tiles.append(t)
        return tiles

    qTs = [sb.tile([P, T], BF16, name=f"qTs{h}") for h in range(NC)]
    v_sb = [sb.tile([P, T], BF16, name=f"vsb{s}") for s in range(NC)]

    def proj_qk(w_t, x_in, x_out):
        for ht in range(NC):
            for tg in range(2):
                acc = ps.tile([P, 512], F32, name="ps")
                for ec in range(NC):
                    nc.tensor.matmul(
                        acc[:],
                        lhsT=w_t[ec][:, bass.ts(ht, P)],
                        rhs=x_in[ec][:, bass.ts(tg, 512)],
                        start=(ec == 0),
                        stop=(ec == NC - 1),
                    )
                nc.scalar.copy(x_out[ht][:, bass.ts(tg, 512)], acc[:])

    qin = load_in("qin", "qT", interleave=pools.pop("wq_dma", None))
    proj_qk(w_q, qin, qTs)
    # raw key^T chunks feed the score matmuls directly (k-proj folded into W)
    kTs = load_in("kTs", "kT", interleave=pools.pop("wv_dma", None))
    vin = load_in("vin", "vT")
    padt = sb.tile([P, NC], BF16, name="padt", bufs=2)
    nc.sync.dma_start(
        padt[:], dram["pad"][b].rearrange("(c p) x -> p (c x)", p=P)
    )
    for st in range(NC):
        for hh in range(2):
            acc = ps.tile([P, 512], F32, name="ps")
            for ec in range(NC):
                nc.tensor.matmul(
                    acc[:],
                    lhsT=vin[ec][:, bass.ts(st, P)],
                    rhs=w_v[ec][:, bass.ts(hh, 512)],
                    start=(ec == 0),
                    stop=(ec == NC - 1),
                )
            nc.vector.tensor_copy(v_sb[st][:, bass.ts(hh, 512)], acc[:])

    # -- scores^T + exp + causal zeroing --
    # 256-wide t-groups; the straddle block (off == 256) is computed at
    # half width (only its second t-half is causally live), which brings
    # the block count to the causal minimum 43.
    GW = 256
    pT = [sb.tile([P, T], BF16, name=f"pT{s}") for s in range(NC)]
    for g in range(T // GW):
        for sc in range(min((GW * (g + 1)) // P + 1, NC)):
            off = 128 * sc - GW * g
            # column window of this t-group that is causally live
            c0 = 128 if off >= 256 else 0
            cw = GW - c0
            acc = ps.tile([P, 512], F32, name="ps")
            for hc in range(NC):
                nc.tensor.matmul(
                    acc[:, :cw],
                    lhsT=kTs[hc][:, bass.ts(sc, P)],
                    rhs=qTs[hc][:, GW * g + c0: GW * g + GW],
                    start=(hc == 0),
                    stop=(hc == NC - 1),
                )
            dst = pT[sc][:, GW * g + c0: GW * g + GW]
            nc.scalar.activation(dst, acc[:, :cw], Exp, scale=SCALE)
            if off >= 0:
                # keep where t_local - s_local - (off-c0) + 1 >= 0 (j <= i+1)
                nc.gpsimd.affine_select(
                    out=dst,
                    in_=dst,
                    compare_op=mybir.AluOpType.is_ge,
                    fill=0.0,
                    base=1 - (off - c0),
                    pattern=[[1, cw]],
                    channel_multiplier=-1,
                )

    # -- attn = (pT.T @ [v, pad01]) with post-normalization --
    for ti in range(NC):
        nsc = _n_sc(ti)
        po0 = ps.tile([P, 512], F32, name="ps")
        po1 = ps.tile([P, 512], F32, name="ps")
        pd = psd.tile([P, 1], F32, name="psd")
        for sc in range(nsc):
            lhsT = pT[sc][:, bass.ts(ti, P)]
            st, sp = (sc == 0), (sc == nsc - 1)
            # den (N=1) sits between the two wide matmuls so its ldweights
            # and po1's both hide under po0's 512-cycle moving phase
            nc.tensor.matmul(po0[:], lhsT=lhsT, rhs=v_sb[sc][:, 0:512],
                             start=st, stop=sp)
            nc.tensor.matmul(pd[:], lhsT=lhsT, rhs=padt[:, sc:sc + 1],
                             start=st, stop=sp)
            nc.tensor.matmul(po1[:], lhsT=lhsT, rhs=v_sb[sc][:, 512:1024],
                             start=st, stop=sp)
        r = sb.tile([P, 1], F32, name="recip", bufs=3)
        nc.vector.reciprocal(r[:], pd[:])
        osb = sb.tile([P, T], BF16, name="osb", bufs=3)
        # the two halves scale concurrently on VectorE and ScalarE
        nc.vector.tensor_scalar_mul(osb[:, 0:512], po0[:], r[:])
        nc.sync.dma_start(dram["out"][b, bass.ts(ti, P), 0:512], osb[:, 0:512])
        nc.scalar.activation(osb[:, 512:1024], po1[:],
                             mybir.ActivationFunctionType.Copy, scale=r[:])
        nc.sync.dma_start(dram["out"][b, bass.ts(ti, P), 512:1024],
                          osb[:, 512:1024])


def _build_nc():
    nc = bass.Bass()
    dram = {
        "qT": nc.declare_dram_parameter("qT", [BPC, E, T], BF16, isOutput=False),
        "kT": nc.declare_dram_parameter("kT", [BPC, E, T], BF16, isOutput=False),
        "vT": nc.declare_dram_parameter("vT", [BPC, E, T], BF16, isOutput=False),
        # "wq" holds W = Wq.T @ Wk (k-proj folded on host)
        "wq": nc.declare_dram_parameter("wq", [E, H], BF16, isOutput=False),
        "wv": nc.declare_dram_parameter("wv", [E, H], BF16, isOutput=False),
        "pad": nc.declare_dram_parameter("pad", [BPC, T, 1], BF16, isOutput=False),
        "out": nc.declare_dram_parameter("out", [BPC, T, H], BF16, isOutput=True),
    }
    with tile.TileContext(nc) as tc, ExitStack() as ctx:
        sb = ctx.enter_context(tc.tile_pool(name="sb", bufs=1))
        ps = ctx.enter_context(tc.tile_pool(name="ps", bufs=6, space="PSUM"))
        psd = ctx.enter_context(tc.tile_pool(name="psd", bufs=2, space="PSUM"))

        pools = {"sb": sb, "ps": ps, "psd": psd}
        for wname in ("wq", "wv"):
            pools[wname] = [
                sb.tile([P, H], BF16, name=f"{wname}{ec}") for ec in range(NC)
            ]

        def w_dma(wname):
            def go(ec):
                nc.sync.dma_start(
                    pools[wname][ec][:], dram[wname][bass.ts(ec, P), :]
                )
            return go

        # Weight DMAs interleave chunk-by-chunk with batch 0's input loads.
        pools["wq_dma"] = w_dma("wq")
        pools["wv_dma"] = w_dma("wv")

        # No junk warm-up: the first projection's matmuls are DMA-paced and
        # run during the HAM clock ramp themselves (real work at slow clock
        # beats junk work at slow clock).

        for b in range(BPC):
            _emit_batch(nc, pools, b, dram)

    _split_multi_waits(nc)
    return nc


def _get_nc():
    global _nc_cache
    if _nc_cache is None:
        _nc_cache = _build_nc()
    return _nc_cache


def _make_in_maps(key, query, value, padding_mask, Wk, Wq, Wv):
    bf = ml_dtypes.bfloat16
    # Fold the k-projection into the q side: q @ k.T = query @ W @ key.T
    W = (Wq.astype(np.float64).T @ Wk.astype(np.float64)).astype(np.float32)
    wq = np.ascontiguousarray(W).astype(bf)  # [E, E]
    wv = np.ascontiguousarray(Wv.T).astype(bf)
    pad01 = (padding_mask.reshape(NB, T) == 0).astype(np.float32)  # [B,T]
    in_maps = []
    for c in range(NCORES):
        s = slice(BPC * c, BPC * (c + 1))
        qT = np.ascontiguousarray(query[s].transpose(0, 2, 1)).astype(bf)
        kT = np.ascontiguousarray(key[s].transpose(0, 2, 1)).astype(bf)
        vTf = value[s].transpose(0, 2, 1) * pad01[s][:, None, :]
        vT = np.ascontiguousarray(vTf).astype(bf)
        in_maps.append({
            "qT": qT, "kT": kT, "vT": vT,
            "wq": wq, "wv": wv,
            "pad": pad01[s].astype(bf).reshape(BPC, T, 1),
        })
    return in_maps


def run_on_cores(in_maps, trace=False, **kw):
    nc = _get_nc()
    return run_bass_kernel_spmd(nc, in_maps, list(range(NCORES)), trace=trace, **kw)


def kernel(key, query, value, padding_mask, Wk, Wq, Wv):
    key = np.asarray(key)
    query = np.asarray(query)
    value = np.asarray(value)
    padding_mask = np.asarray(padding_mask)
    in_maps = _make_in_maps(key, query, value, padding_mask,
                            np.asarray(Wk), np.asarray(Wq), np.asarray(Wv))
    res = run_on_cores(in_maps)
    out = np.empty((NB, T, H), np.float32)
    for c in range(NCORES):
        out[BPC * c: BPC * (c + 1)] = res.results[c]["out"].astype(np.float32)
    return out

